# revision 35
# baseline (speedup 1.0000x reference)
"""Single-head attention (B=8, S=2048, IN=1024, QD=128, VD=1024) on 8 TRN2
NeuronCores, data-parallel over batch (one batch element per core).

Math per core (batch b):
    q = x Wq + bq ; k = x Wk + bk ; v = x Wv + bv
    out = tanh(softmax(q k^T) v)

Layout strategy (all matmuls contract over the partition dim):
  - host pre-transposes x[b] -> xT [IN, S] so projections need no on-chip
    transpose. qT [QD, S] = Wq^T xT, kT likewise, v [S, VD] = xT^T Wv.
  - scores are built TRANSPOSED: sT [t, s] = kT^T qT, so exp(sT) ("E^T")
    is directly the stationary operand of the AV matmul:
        o [s, VD] = (E^T)^T v   (accumulated over 16 t-tiles in PSUM)
    and softmax needs no max-subtraction (|scores| <= ~21, exp is finite
    in fp32) and no transposes.
  - row-denominators come from an extra N=1 matmul per (s,t) tile with an
    all-ones rhs; normalization folds into the final tanh activation as a
    per-partition scale: out = tanh(o_raw * recip(denom)).

Dtypes: q/k/v/scores matmuls run in float32r (fp32 layout, ~11-bit mantissa
rounding on HW, 1 cycle/row vs fp32's 4); E and the AV matmul run in bf16.
Measured: absmax error vs fp32 reference 6.5e-3 (scale ~1), HW exec time
~244 us/core (PE issue-limited end to end, ~79% of 78.6 TF/s peak).

Pipelining: (1) phases B1/B2 run as two kt-half passes (PSUM partials
drained to SBUF, second pass adds in place) so the v-projection's first
half overlaps the xt4-7/wv DMA stream and the PE never stalls on input
DMA after its first tile; (2) phase C interleaves block n's scores+exp
(per t-tile) into block n-1's first AV accumulation loop so the PE array
never sees a low-density stretch (keeps the HAM clock gate at K=8/8);
block 0's scores ride inside the v-projection tail.
"""

import numpy as np

import concourse.bacc as bacc
import concourse.mybir as mybir
import concourse.tile as tile
from concourse.bass_utils import run_bass_kernel_spmd

B, S, IN, QD, VD = 8, 2048, 1024, 128, 1024
N_CORES = 8
P = 128
KT = IN // P          # 8 contraction tiles for projections
TT = S // P           # 16 t-tiles
S_BLK = 256           # s-block width for scores/E^T staging
N_BLK = S // S_BLK    # 8 blocks
SS = S_BLK // P       # 2 s-subtiles per block

F32 = mybir.dt.float32
F32R = mybir.dt.float32r
BF16 = mybir.dt.bfloat16

_CACHE: dict = {}


def _build():
    if "nc" in _CACHE:
        return _CACHE["nc"]

    nc = bacc.Bacc("TRN2", target_bir_lowering=False, debug=False,
                   num_devices=N_CORES)

    xT_d = nc.dram_tensor("xT", [IN, S], F32, kind="ExternalInput").ap()
    wq_d = nc.dram_tensor("wq", [P, KT, QD], F32, kind="ExternalInput").ap()
    wk_d = nc.dram_tensor("wk", [P, KT, QD], F32, kind="ExternalInput").ap()
    wv_d = nc.dram_tensor("wv", [P, KT, VD], F32, kind="ExternalInput").ap()
    bq_d = nc.dram_tensor("bq", [QD], F32, kind="ExternalInput").ap()
    bk_d = nc.dram_tensor("bk", [QD], F32, kind="ExternalInput").ap()
    bv_d = nc.dram_tensor("bv", [VD], F32, kind="ExternalInput").ap()
    out_d = nc.dram_tensor("out", [S, VD], F32, kind="ExternalOutput").ap()
    wu_d = nc.dram_tensor("wu_scratch", [P, 512], F32)

    with tile.TileContext(nc) as tc:
        with (
            tc.tile_pool(name="consts", bufs=1) as consts,
            tc.tile_pool(name="xt", bufs=KT) as p_xt,
            tc.tile_pool(name="wv", bufs=KT) as p_wv,
            tc.tile_pool(name="qk", bufs=1) as p_qk,
            tc.tile_pool(name="v", bufs=TT) as p_v,
            tc.tile_pool(name="et", bufs=2 * TT) as p_et,
            tc.tile_pool(name="o", bufs=4) as p_o,
            tc.tile_pool(name="recip", bufs=4) as p_recip,
            tc.tile_pool(name="ps", bufs=8, space="PSUM") as ps,
        ):
            # ---- PE warm-up ----
            # The PE sits idle from the ~6us preamble end until the first
            # projection inputs land (~19us). Dummy matmuls on memset tiles
            # (no DMA deps) keep the HAM activity monitor busy so the real
            # matmul stream starts at 2.4 GHz instead of ramping from 1.2.
            # The accumulated result is DMA'd to a DRAM scratch so DCE
            # keeps the chain.
            wu_lhs = consts.tile([1, P], BF16, tag="wu_lhs")
            wu_rhs = consts.tile([1, 512], BF16, tag="wu_rhs")
            nc.vector.memset(wu_lhs[:], 0.0)
            nc.vector.memset(wu_rhs[:], 0.0)
            N_WU = 56
            wu_ps = ps.tile([P, 512], F32, tag="ps", name="wu_ps")
            for i in range(N_WU):
                nc.tensor.matmul(wu_ps[:], wu_lhs[:], wu_rhs[:],
                                 start=(i == 0), stop=(i == N_WU - 1))
            wu_sb = consts.tile([P, 512], F32, tag="wu_sb")
            nc.vector.tensor_copy(wu_sb[:], wu_ps[:])
            # (wu_sb -> DRAM DMA is emitted at the very end of the program:
            # emitting it here would head-of-line-block the input loads in
            # the HWDGE FIFO behind the warm-up chain)

            # ---- constant / weight loads ----
            wq_sb = consts.tile([P, KT, QD], F32R, tag="wq")
            wk_sb = consts.tile([P, KT, QD], F32R, tag="wk")
            nc.sync.dma_start(out=wq_sb[:], in_=wq_d.bitcast(F32R))
            nc.sync.dma_start(out=wk_sb[:], in_=wk_d.bitcast(F32R))

            def wq_at(kt):
                return wq_sb[:, kt, :]

            def wk_at(kt):
                return wk_sb[:, kt, :]

            ones_sb = consts.tile([P, 1], BF16, tag="ones")
            nc.vector.memset(ones_sb[:], 1.0)

            xt_sb = []
            bq_sb = consts.tile([P, 1], F32, tag="bq")
            bk_sb = consts.tile([P, 1], F32, tag="bk")
            bv_row = consts.tile([1, VD], F32, tag="bv_row")
            bv_sb = consts.tile([P, VD], F32, tag="bv")
            wv_sb = [None] * KT

            def load_wv(kt):
                t_ = p_wv.tile([P, VD], F32R, tag="wv", name=f"wvt{kt}")
                nc.sync.dma_start(out=t_[:], in_=wv_d[:, kt, :].bitcast(F32R))
                wv_sb[kt] = t_

            for kt in range(KT):
                t_ = p_xt.tile([P, S], F32R, tag="xt", name=f"xt{kt}")
                nc.sync.dma_start(out=t_[:],
                                  in_=xT_d[kt * P:(kt + 1) * P, :].bitcast(F32R))
                xt_sb.append(t_)
                if kt == KT // 2 - 1:
                    # wv0-3 between xt3 and xt4: needed by the first
                    # v-projection half-pass
                    for wkt in range(KT // 2):
                        load_wv(wkt)
                if kt == 0:
                    # small loads tucked behind xt0 so they don't delay it
                    # but still land long before their first use
                    nc.sync.dma_start(out=bq_sb[:],
                                      in_=bq_d.rearrange("(p o) -> p o", o=1))
                    nc.sync.dma_start(out=bk_sb[:],
                                      in_=bk_d.rearrange("(p o) -> p o", o=1))
                    nc.sync.dma_start(out=bv_row[:],
                                      in_=bv_d.rearrange("(o v) -> o v", o=1))
                    # broadcast bv across partitions via a K=1 outer product
                    # (ones[1,P] x bv[1,VD]) - no HBM bandwidth stolen from
                    # the xt/wv input stream
                    ones_row = consts.tile([1, P], BF16, tag="ones_row")
                    nc.vector.memset(ones_row[:], 1.0)
                    bv_row_bf = consts.tile([1, VD], BF16, tag="bv_row_bf")
                    nc.vector.tensor_copy(bv_row_bf[:], bv_row[:])
                    for c in range(VD // 512):
                        bv_ps = ps.tile([P, 512], F32, tag="ps",
                                        name=f"bvps{c}")
                        nc.tensor.matmul(bv_ps[:], ones_row[:],
                                         bv_row_bf[:, c * 512:(c + 1) * 512],
                                         start=True, stop=True)
                        nc.vector.tensor_copy(bv_sb[:, c * 512:(c + 1) * 512],
                                              bv_ps[:])

            # ---- phases B1/B2 as two kt-half passes ----
            # Projections accumulate kt 0-3 into PSUM, drain partials to
            # SBUF, then a second pass adds kt 4-7 in place. Freeing all 8
            # PSUM banks between passes lets the v-projection's first half
            # (which only needs xt0-3 + wv0-3) run while xt4-7 / wv4-7 are
            # still streaming in, so the PE never waits on the input DMA
            # after its first tile.
            qT_sb = p_qk.tile([P, S], F32R, tag="qT")
            kT_sb = p_qk.tile([P, S], F32R, tag="kT")
            NSC = S // 512  # 4
            NVC = VD // 512
            KH = KT // 2

            def proj_pass(half):
                k0 = half * KH
                q_ps = [ps.tile([P, 512], F32, tag="ps",
                                name=f"qps{half}_{i}") for i in range(NSC)]
                k_ps = [ps.tile([P, 512], F32, tag="ps",
                                name=f"kps{half}_{i}") for i in range(NSC)]
                for kt in range(k0, k0 + KH):
                    # all q chunks then all k chunks: one weight load per
                    # group instead of one per matmul
                    for sc in range(NSC):
                        nc.tensor.matmul(q_ps[sc][:], wq_at(kt),
                                         xt_sb[kt][:, sc * 512:(sc + 1) * 512],
                                         start=(kt == k0),
                                         stop=(kt == k0 + KH - 1))
                    for sc in range(NSC):
                        nc.tensor.matmul(k_ps[sc][:], wk_at(kt),
                                         xt_sb[kt][:, sc * 512:(sc + 1) * 512],
                                         start=(kt == k0),
                                         stop=(kt == k0 + KH - 1))
                for sc in range(NSC):
                    sl = slice(sc * 512, (sc + 1) * 512)
                    if half == 0:
                        nc.vector.tensor_scalar_add(qT_sb[:, sl], q_ps[sc][:],
                                                    bq_sb[:])
                        nc.vector.tensor_scalar_add(kT_sb[:, sl], k_ps[sc][:],
                                                    bk_sb[:])
                    else:
                        nc.vector.tensor_add(qT_sb[:, sl], q_ps[sc][:],
                                             qT_sb[:, sl])
                        nc.vector.tensor_add(kT_sb[:, sl], k_ps[sc][:],
                                             kT_sb[:, sl])

            proj_pass(0)

            # ---- phase C helper (defined early: scores for block 0 are
            # interleaved into phase B2's tail) ----
            def emit_scores_t(sb, t):
                s0 = sb * S_BLK
                st_ps = ps.tile([P, S_BLK], F32, tag="ps", name=f"stps{sb}_{t}")
                nc.tensor.matmul(st_ps[:],
                                 kT_sb[:, t * P:(t + 1) * P],
                                 qT_sb[:, s0:s0 + S_BLK],
                                 start=True, stop=True)
                et = p_et.tile([P, S_BLK], BF16, tag="et", name=f"et{sb}_{t}")
                nc.scalar.activation(out=et[:], in_=st_ps[:],
                                     func=mybir.ActivationFunctionType.Exp)
                return et

            # ---- phase B2: v [S, VD] = xT^T Wv + bv, stored bf16 ----
            # Two kt-half passes; pass 0 stores bf16 partials (+bv) in v_sb,
            # pass 1 adds the kt 4-7 contribution in place. Block 0's 16
            # scores/exp tiles ride along in pass 1's last iterations so
            # phase C starts with E^T(0) already staged.
            v_sb = [p_v.tile([P, VD], BF16, tag="v", name=f"v{t}")
                    for t in range(TT)]
            et0 = []

            def v_pass(half, interleave0):
                k0 = half * KH
                for t in range(TT):
                    vt = v_sb[t]
                    if interleave0 and t >= TT - 8:
                        et0.append(emit_scores_t(0, len(et0)))
                    v_ps = [ps.tile([P, 512], F32, tag="ps",
                                    name=f"vps{half}_{t}_{vc}")
                            for vc in range(NVC)]
                    for kt in range(k0, k0 + KH):
                        xl = xt_sb[kt][:, t * P:(t + 1) * P]
                        for vc in range(NVC):
                            nc.tensor.matmul(
                                v_ps[vc][:], xl,
                                wv_sb[kt][:, vc * 512:(vc + 1) * 512],
                                start=(kt == k0), stop=(kt == k0 + KH - 1))
                    if interleave0 and t >= TT - 8:
                        et0.append(emit_scores_t(0, len(et0)))
                    for vc in range(NVC):
                        sl = slice(vc * 512, (vc + 1) * 512)
                        if half == 0:
                            nc.vector.tensor_add(vt[:, sl], v_ps[vc][:],
                                                 bv_sb[:, sl])
                        else:
                            nc.vector.tensor_add(vt[:, sl], v_ps[vc][:],
                                                 vt[:, sl])

            v_pass(0, interleave0=False)
            for kt in range(KT // 2, KT):
                load_wv(kt)
            proj_pass(1)
            v_pass(1, interleave0=True)

            # ---- phase C: software-pipelined over s-blocks ----
            # Block n's scores^T + exp are interleaved (per t) into block
            # n-1's first AV accumulation loop so the PE array never sees a
            # low-density stretch (keeps HAM at K=8/8) and exp latency hides
            # under the AV matmul stream.
            def emit_av_ss(sb, ss, et_tiles, interleave_sb=None,
                           serialize_vc=False):
                # One AV accumulation group (128 output rows x full VD) plus
                # its denominator; optionally interleaves the next block's
                # scores/exp into the t loop.
                o_ps = [ps.tile([P, 512], F32, tag="ps", name=f"ops{sb}_{ss}_{i}")
                        for i in range(VD // 512)]
                d_ps = ps.tile([P, 1], F32, tag="ps", name=f"dps{sb}_{ss}")
                nxt = []
                recip = p_recip.tile([P, 1], F32, tag="recip",
                                     name=f"recip{sb}_{ss}")
                o_sb = p_o.tile([P, VD], F32, tag="o", name=f"osb{sb}_{ss}")
                srow = sb * S_BLK + ss * P

                def drain_vc(vc):
                    nc.scalar.activation(
                        out=o_sb[:, vc * 512:(vc + 1) * 512],
                        in_=o_ps[vc][:],
                        func=mybir.ActivationFunctionType.Tanh,
                        scale=recip[:])
                    nc.sync.dma_start(
                        out=out_d[srow:srow + P, vc * 512:(vc + 1) * 512],
                        in_=o_sb[:, vc * 512:(vc + 1) * 512])

                if not serialize_vc:
                    for t in range(TT):
                        if interleave_sb is not None:
                            nxt.append(emit_scores_t(interleave_sb, t))
                        lhs = et_tiles[t][:, ss * P:(ss + 1) * P]
                        for vc in range(VD // 512):
                            nc.tensor.matmul(o_ps[vc][:], lhs,
                                             v_sb[t][:, vc * 512:(vc + 1) * 512],
                                             start=(t == 0), stop=(t == TT - 1))
                        nc.tensor.matmul(d_ps[:], lhs, ones_sb[:],
                                         start=(t == 0), stop=(t == TT - 1))
                    nc.vector.reciprocal(recip[:], d_ps[:])
                    for vc in range(VD // 512):
                        drain_vc(vc)
                else:
                    # tail variant: finish vc0 (and the denominator) first so
                    # its tanh+DMA overlap vc1's accumulation
                    for t in range(TT):
                        lhs = et_tiles[t][:, ss * P:(ss + 1) * P]
                        nc.tensor.matmul(o_ps[0][:], lhs, v_sb[t][:, 0:512],
                                         start=(t == 0), stop=(t == TT - 1))
                        nc.tensor.matmul(d_ps[:], lhs, ones_sb[:],
                                         start=(t == 0), stop=(t == TT - 1))
                    nc.vector.reciprocal(recip[:], d_ps[:])
                    drain_vc(0)
                    for t in range(TT):
                        lhs = et_tiles[t][:, ss * P:(ss + 1) * P]
                        nc.tensor.matmul(o_ps[1][:], lhs,
                                         v_sb[t][:, 512:1024],
                                         start=(t == 0), stop=(t == TT - 1))
                    drain_vc(1)
                return nxt

            et_cur = et0
            for sb in range(N_BLK):
                nxt_sb = sb + 1 if sb + 1 < N_BLK else None
                et_nxt = emit_av_ss(sb, 0, et_cur, interleave_sb=nxt_sb)
                emit_av_ss(sb, 1, et_cur,
                           serialize_vc=(sb == N_BLK - 1))
                et_cur = et_nxt

            nc.sync.dma_start(out=wu_d.ap(), in_=wu_sb[:])

    nc.compile()
    _CACHE["nc"] = nc
    return nc


def _prep_inputs(x, Wq, bq, Wk, bk, Wv, bv):
    x = np.asarray(x, np.float32)
    xT = np.ascontiguousarray(x.transpose(0, 2, 1))          # [B, IN, S]
    wq = np.ascontiguousarray(
        np.asarray(Wq, np.float32).reshape(KT, P, QD).transpose(1, 0, 2))
    wk = np.ascontiguousarray(
        np.asarray(Wk, np.float32).reshape(KT, P, QD).transpose(1, 0, 2))
    wv = np.ascontiguousarray(
        np.asarray(Wv, np.float32).reshape(KT, P, VD).transpose(1, 0, 2))
    shared = {
        "wq": wq, "wk": wk, "wv": wv,
        "bq": np.asarray(bq, np.float32),
        "bk": np.asarray(bk, np.float32),
        "bv": np.asarray(bv, np.float32),
    }
    return [dict(shared, xT=xT[c]) for c in range(N_CORES)]


def run(x, Wq, bq, Wk, bk, Wv, bv, trace=False):
    nc = _build()
    in_maps = _prep_inputs(x, Wq, bq, Wk, bk, Wv, bv)
    res = run_bass_kernel_spmd(nc, in_maps, list(range(N_CORES)), trace=trace)
    out = np.stack([res.results[c]["out"] for c in range(N_CORES)])
    return out.astype(np.float32), res


def kernel(x, Wq, bq, Wk, bk, Wv, bv):
    out, _ = run(x, Wq, bq, Wk, bk, Wv, bv, trace=False)
    return out


# revision 36
# speedup vs baseline: 1.0607x; 1.0607x over previous
"""Single-head attention (B=8, S=2048, IN=1024, QD=128, VD=1024) on 8 TRN2
NeuronCores, data-parallel over batch (one batch element per core).

Math per core (batch b):
    q = x Wq + bq ; k = x Wk + bk ; v = x Wv + bv
    out = tanh(softmax(q k^T) v)

Layout strategy (all matmuls contract over the partition dim):
  - host pre-transposes x[b] -> xT [IN, S] so projections need no on-chip
    transpose. qT [QD, S] = Wq^T xT, kT likewise, v [S, VD] = xT^T Wv.
  - scores are built TRANSPOSED: sT [t, s] = kT^T qT, so exp(sT) ("E^T")
    is directly the stationary operand of the AV matmul:
        o [s, VD] = (E^T)^T v   (accumulated over 16 t-tiles in PSUM)
    and softmax needs no max-subtraction (|scores| <= ~21, exp is finite
    in fp32) and no transposes.
  - row-denominators come from an extra N=1 matmul per (s,t) tile with an
    all-ones rhs; normalization folds into the final tanh activation as a
    per-partition scale: out = tanh(o_raw * recip(denom)).

Dtypes: q/k/v/scores matmuls run in float32r (fp32 layout, ~11-bit mantissa
rounding on HW, 1 cycle/row vs fp32's 4); E and the AV matmul run in bf16.
Measured: absmax error vs fp32 reference 6.5e-3 (scale ~1), HW exec time
~244 us/core (PE issue-limited end to end, ~79% of 78.6 TF/s peak).

Pipelining: (1) phases B1/B2 run as two kt-half passes (PSUM partials
drained to SBUF, second pass adds in place) so the v-projection's first
half overlaps the xt4-7/wv DMA stream and the PE never stalls on input
DMA after its first tile; (2) phase C interleaves block n's scores+exp
(per t-tile) into block n-1's first AV accumulation loop so the PE array
never sees a low-density stretch (keeps the HAM clock gate at K=8/8);
block 0's scores ride inside the v-projection tail.
"""

import numpy as np

import concourse.bacc as bacc
import concourse.mybir as mybir
import concourse.tile as tile
from concourse.bass_utils import run_bass_kernel_spmd

B, S, IN, QD, VD = 8, 2048, 1024, 128, 1024
N_CORES = 8
P = 128
KT = IN // P          # 8 contraction tiles for projections
TT = S // P           # 16 t-tiles
S_BLK = 256           # s-block width for scores/E^T staging
N_BLK = S // S_BLK    # 8 blocks
SS = S_BLK // P       # 2 s-subtiles per block

F32 = mybir.dt.float32
F32R = mybir.dt.float32r
BF16 = mybir.dt.bfloat16

_CACHE: dict = {}


def _build():
    if "nc" in _CACHE:
        return _CACHE["nc"]

    nc = bacc.Bacc("TRN2", target_bir_lowering=False, debug=False,
                   num_devices=N_CORES)

    xT_d = nc.dram_tensor("xT", [IN, S], F32, kind="ExternalInput").ap()
    wq_d = nc.dram_tensor("wq", [P, KT, QD], F32, kind="ExternalInput").ap()
    wk_d = nc.dram_tensor("wk", [P, KT, QD], F32, kind="ExternalInput").ap()
    wv_d = nc.dram_tensor("wv", [P, KT, VD], F32, kind="ExternalInput").ap()
    bq_d = nc.dram_tensor("bq", [QD], F32, kind="ExternalInput").ap()
    bk_d = nc.dram_tensor("bk", [QD], F32, kind="ExternalInput").ap()
    bv_d = nc.dram_tensor("bv", [VD], F32, kind="ExternalInput").ap()
    out_d = nc.dram_tensor("out", [S, VD], F32, kind="ExternalOutput").ap()

    with tile.TileContext(nc) as tc:
        with (
            tc.tile_pool(name="consts", bufs=1) as consts,
            tc.tile_pool(name="xt", bufs=KT) as p_xt,
            tc.tile_pool(name="wv", bufs=KT) as p_wv,
            tc.tile_pool(name="qk", bufs=1) as p_qk,
            tc.tile_pool(name="v", bufs=TT) as p_v,
            tc.tile_pool(name="et", bufs=2 * TT) as p_et,
            tc.tile_pool(name="o", bufs=4) as p_o,
            tc.tile_pool(name="recip", bufs=4) as p_recip,
            tc.tile_pool(name="ps", bufs=8, space="PSUM") as ps,
        ):
            # ---- constant / weight loads ----
            wq_sb = consts.tile([P, KT, QD], F32R, tag="wq")
            wk_sb = consts.tile([P, KT, QD], F32R, tag="wk")
            nc.sync.dma_start(out=wq_sb[:], in_=wq_d.bitcast(F32R))
            nc.sync.dma_start(out=wk_sb[:], in_=wk_d.bitcast(F32R))

            def wq_at(kt):
                return wq_sb[:, kt, :]

            def wk_at(kt):
                return wk_sb[:, kt, :]

            ones_sb = consts.tile([P, 1], BF16, tag="ones")
            nc.vector.memset(ones_sb[:], 1.0)

            xt_sb = []
            bq_sb = consts.tile([P, 1], F32, tag="bq")
            bk_sb = consts.tile([P, 1], F32, tag="bk")
            bv_row = consts.tile([1, VD], F32, tag="bv_row")
            bv_sb = consts.tile([P, VD], F32, tag="bv")
            wv_sb = [None] * KT

            def load_wv(kt):
                t_ = p_wv.tile([P, VD], F32R, tag="wv", name=f"wvt{kt}")
                nc.sync.dma_start(out=t_[:], in_=wv_d[:, kt, :].bitcast(F32R))
                wv_sb[kt] = t_

            for kt in range(KT):
                t_ = p_xt.tile([P, S], F32R, tag="xt", name=f"xt{kt}")
                nc.sync.dma_start(out=t_[:],
                                  in_=xT_d[kt * P:(kt + 1) * P, :].bitcast(F32R))
                xt_sb.append(t_)
                if kt == KT // 2 - 1:
                    # wv0-3 between xt3 and xt4: needed by the first
                    # v-projection half-pass
                    for wkt in range(KT // 2):
                        load_wv(wkt)
                if kt == 0:
                    # small loads tucked behind xt0 so they don't delay it
                    # but still land long before their first use
                    nc.sync.dma_start(out=bq_sb[:],
                                      in_=bq_d.rearrange("(p o) -> p o", o=1))
                    nc.sync.dma_start(out=bk_sb[:],
                                      in_=bk_d.rearrange("(p o) -> p o", o=1))
                    nc.sync.dma_start(out=bv_row[:],
                                      in_=bv_d.rearrange("(o v) -> o v", o=1))
                    # broadcast bv across partitions via a K=1 outer product
                    # (ones[1,P] x bv[1,VD]) - no HBM bandwidth stolen from
                    # the xt/wv input stream
                    ones_row = consts.tile([1, P], BF16, tag="ones_row")
                    nc.vector.memset(ones_row[:], 1.0)
                    bv_row_bf = consts.tile([1, VD], BF16, tag="bv_row_bf")
                    nc.vector.tensor_copy(bv_row_bf[:], bv_row[:])
                    for c in range(VD // 512):
                        bv_ps = ps.tile([P, 512], F32, tag="ps",
                                        name=f"bvps{c}")
                        nc.tensor.matmul(bv_ps[:], ones_row[:],
                                         bv_row_bf[:, c * 512:(c + 1) * 512],
                                         start=True, stop=True)
                        nc.vector.tensor_copy(bv_sb[:, c * 512:(c + 1) * 512],
                                              bv_ps[:])

            # ---- phases B1/B2 as two kt-half passes ----
            # Projections accumulate kt 0-3 into PSUM, drain partials to
            # SBUF, then a second pass adds kt 4-7 in place. Freeing all 8
            # PSUM banks between passes lets the v-projection's first half
            # (which only needs xt0-3 + wv0-3) run while xt4-7 / wv4-7 are
            # still streaming in, so the PE never waits on the input DMA
            # after its first tile.
            qT_sb = p_qk.tile([P, S], F32R, tag="qT")
            kT_sb = p_qk.tile([P, S], F32R, tag="kT")
            NSC = S // 512  # 4
            NVC = VD // 512
            KH = KT // 2

            def proj_pass(half):
                k0 = half * KH
                q_ps = [ps.tile([P, 512], F32, tag="ps",
                                name=f"qps{half}_{i}") for i in range(NSC)]
                k_ps = [ps.tile([P, 512], F32, tag="ps",
                                name=f"kps{half}_{i}") for i in range(NSC)]
                for kt in range(k0, k0 + KH):
                    # all q chunks then all k chunks: one weight load per
                    # group instead of one per matmul
                    for sc in range(NSC):
                        nc.tensor.matmul(q_ps[sc][:], wq_at(kt),
                                         xt_sb[kt][:, sc * 512:(sc + 1) * 512],
                                         start=(kt == k0),
                                         stop=(kt == k0 + KH - 1))
                    for sc in range(NSC):
                        nc.tensor.matmul(k_ps[sc][:], wk_at(kt),
                                         xt_sb[kt][:, sc * 512:(sc + 1) * 512],
                                         start=(kt == k0),
                                         stop=(kt == k0 + KH - 1))
                for sc in range(NSC):
                    sl = slice(sc * 512, (sc + 1) * 512)
                    if half == 0:
                        nc.vector.tensor_scalar_add(qT_sb[:, sl], q_ps[sc][:],
                                                    bq_sb[:])
                        nc.vector.tensor_scalar_add(kT_sb[:, sl], k_ps[sc][:],
                                                    bk_sb[:])
                    else:
                        nc.vector.tensor_add(qT_sb[:, sl], q_ps[sc][:],
                                             qT_sb[:, sl])
                        nc.vector.tensor_add(kT_sb[:, sl], k_ps[sc][:],
                                             kT_sb[:, sl])

            proj_pass(0)

            # ---- phase C helper (defined early: scores for block 0 are
            # interleaved into phase B2's tail) ----
            def emit_scores_t(sb, t):
                s0 = sb * S_BLK
                st_ps = ps.tile([P, S_BLK], F32, tag="ps", name=f"stps{sb}_{t}")
                nc.tensor.matmul(st_ps[:],
                                 kT_sb[:, t * P:(t + 1) * P],
                                 qT_sb[:, s0:s0 + S_BLK],
                                 start=True, stop=True)
                et = p_et.tile([P, S_BLK], BF16, tag="et", name=f"et{sb}_{t}")
                nc.scalar.activation(out=et[:], in_=st_ps[:],
                                     func=mybir.ActivationFunctionType.Exp)
                return et

            # ---- phase B2: v [S, VD] = xT^T Wv + bv, stored bf16 ----
            # Two kt-half passes; pass 0 stores bf16 partials (+bv) in v_sb,
            # pass 1 adds the kt 4-7 contribution in place. Block 0's 16
            # scores/exp tiles ride along in pass 1's last iterations so
            # phase C starts with E^T(0) already staged.
            v_sb = [p_v.tile([P, VD], BF16, tag="v", name=f"v{t}")
                    for t in range(TT)]
            et0 = []

            def v_pass(half, interleave0):
                k0 = half * KH
                for t in range(TT):
                    vt = v_sb[t]
                    if interleave0 and t >= TT - 8:
                        et0.append(emit_scores_t(0, len(et0)))
                    v_ps = [ps.tile([P, 512], F32, tag="ps",
                                    name=f"vps{half}_{t}_{vc}")
                            for vc in range(NVC)]
                    for kt in range(k0, k0 + KH):
                        xl = xt_sb[kt][:, t * P:(t + 1) * P]
                        for vc in range(NVC):
                            nc.tensor.matmul(
                                v_ps[vc][:], xl,
                                wv_sb[kt][:, vc * 512:(vc + 1) * 512],
                                start=(kt == k0), stop=(kt == k0 + KH - 1))
                    if interleave0 and t >= TT - 8:
                        et0.append(emit_scores_t(0, len(et0)))
                    for vc in range(NVC):
                        sl = slice(vc * 512, (vc + 1) * 512)
                        if half == 0:
                            nc.vector.tensor_add(vt[:, sl], v_ps[vc][:],
                                                 bv_sb[:, sl])
                        else:
                            nc.vector.tensor_add(vt[:, sl], v_ps[vc][:],
                                                 vt[:, sl])

            v_pass(0, interleave0=False)
            for kt in range(KT // 2, KT):
                load_wv(kt)
            proj_pass(1)
            v_pass(1, interleave0=True)

            # ---- phase C: software-pipelined over s-blocks ----
            # Block n's scores^T + exp are interleaved (per t) into block
            # n-1's first AV accumulation loop so the PE array never sees a
            # low-density stretch (keeps HAM at K=8/8) and exp latency hides
            # under the AV matmul stream.
            def emit_av_ss(sb, ss, et_tiles, interleave_sb=None,
                           serialize_vc=False):
                # One AV accumulation group (128 output rows x full VD) plus
                # its denominator; optionally interleaves the next block's
                # scores/exp into the t loop.
                o_ps = [ps.tile([P, 512], F32, tag="ps", name=f"ops{sb}_{ss}_{i}")
                        for i in range(VD // 512)]
                d_ps = ps.tile([P, 1], F32, tag="ps", name=f"dps{sb}_{ss}")
                nxt = []
                recip = p_recip.tile([P, 1], F32, tag="recip",
                                     name=f"recip{sb}_{ss}")
                o_sb = p_o.tile([P, VD], F32, tag="o", name=f"osb{sb}_{ss}")
                srow = sb * S_BLK + ss * P

                def drain_vc(vc):
                    nc.scalar.activation(
                        out=o_sb[:, vc * 512:(vc + 1) * 512],
                        in_=o_ps[vc][:],
                        func=mybir.ActivationFunctionType.Tanh,
                        scale=recip[:])
                    nc.sync.dma_start(
                        out=out_d[srow:srow + P, vc * 512:(vc + 1) * 512],
                        in_=o_sb[:, vc * 512:(vc + 1) * 512])

                if not serialize_vc:
                    for t in range(TT):
                        if interleave_sb is not None:
                            nxt.append(emit_scores_t(interleave_sb, t))
                        lhs = et_tiles[t][:, ss * P:(ss + 1) * P]
                        for vc in range(VD // 512):
                            nc.tensor.matmul(o_ps[vc][:], lhs,
                                             v_sb[t][:, vc * 512:(vc + 1) * 512],
                                             start=(t == 0), stop=(t == TT - 1))
                        nc.tensor.matmul(d_ps[:], lhs, ones_sb[:],
                                         start=(t == 0), stop=(t == TT - 1))
                    nc.vector.reciprocal(recip[:], d_ps[:])
                    for vc in range(VD // 512):
                        drain_vc(vc)
                else:
                    # tail variant: finish vc0 (and the denominator) first so
                    # its tanh+DMA overlap vc1's accumulation
                    for t in range(TT):
                        lhs = et_tiles[t][:, ss * P:(ss + 1) * P]
                        nc.tensor.matmul(o_ps[0][:], lhs, v_sb[t][:, 0:512],
                                         start=(t == 0), stop=(t == TT - 1))
                        nc.tensor.matmul(d_ps[:], lhs, ones_sb[:],
                                         start=(t == 0), stop=(t == TT - 1))
                    nc.vector.reciprocal(recip[:], d_ps[:])
                    drain_vc(0)
                    for t in range(TT):
                        lhs = et_tiles[t][:, ss * P:(ss + 1) * P]
                        nc.tensor.matmul(o_ps[1][:], lhs,
                                         v_sb[t][:, 512:1024],
                                         start=(t == 0), stop=(t == TT - 1))
                    drain_vc(1)
                return nxt

            et_cur = et0
            for sb in range(N_BLK):
                nxt_sb = sb + 1 if sb + 1 < N_BLK else None
                et_nxt = emit_av_ss(sb, 0, et_cur, interleave_sb=nxt_sb)
                emit_av_ss(sb, 1, et_cur,
                           serialize_vc=(sb == N_BLK - 1))
                et_cur = et_nxt

    nc.compile()
    _CACHE["nc"] = nc
    return nc


def _prep_inputs(x, Wq, bq, Wk, bk, Wv, bv):
    x = np.asarray(x, np.float32)
    xT = np.ascontiguousarray(x.transpose(0, 2, 1))          # [B, IN, S]
    wq = np.ascontiguousarray(
        np.asarray(Wq, np.float32).reshape(KT, P, QD).transpose(1, 0, 2))
    wk = np.ascontiguousarray(
        np.asarray(Wk, np.float32).reshape(KT, P, QD).transpose(1, 0, 2))
    wv = np.ascontiguousarray(
        np.asarray(Wv, np.float32).reshape(KT, P, VD).transpose(1, 0, 2))
    shared = {
        "wq": wq, "wk": wk, "wv": wv,
        "bq": np.asarray(bq, np.float32),
        "bk": np.asarray(bk, np.float32),
        "bv": np.asarray(bv, np.float32),
    }
    return [dict(shared, xT=xT[c]) for c in range(N_CORES)]


def run(x, Wq, bq, Wk, bk, Wv, bv, trace=False):
    nc = _build()
    in_maps = _prep_inputs(x, Wq, bq, Wk, bk, Wv, bv)
    res = run_bass_kernel_spmd(nc, in_maps, list(range(N_CORES)), trace=trace)
    out = np.stack([res.results[c]["out"] for c in range(N_CORES)])
    return out.astype(np.float32), res


def kernel(x, Wq, bq, Wk, bk, Wv, bv):
    out, _ = run(x, Wq, bq, Wk, bk, Wv, bv, trace=False)
    return out


# revision 37
# speedup vs baseline: 1.0888x; 1.0265x over previous
"""Single-head attention (B=8, S=2048, IN=1024, QD=128, VD=1024) on 8 TRN2
NeuronCores, data-parallel over batch (one batch element per core).

Math per core (batch b):
    q = x Wq + bq ; k = x Wk + bk ; v = x Wv + bv
    out = tanh(softmax(q k^T) v)

Layout strategy (all matmuls contract over the partition dim):
  - host pre-transposes x[b] -> xT [IN, S] so projections need no on-chip
    transpose. qT [QD, S] = Wq^T xT, kT likewise, v [S, VD] = xT^T Wv.
  - scores are built TRANSPOSED: sT [t, s] = kT^T qT, so exp(sT) ("E^T")
    is directly the stationary operand of the AV matmul:
        o [s, VD] = (E^T)^T v   (accumulated over 16 t-tiles in PSUM)
    and softmax needs no max-subtraction (|scores| <= ~21, exp is finite
    in fp32) and no transposes.
  - row-denominators come from an extra N=1 matmul per (s,t) tile with an
    all-ones rhs; normalization folds into the final tanh activation as a
    per-partition scale: out = tanh(o_raw * recip(denom)).

Dtypes: q/k/v/scores matmuls run in float32r (fp32 layout, ~11-bit mantissa
rounding on HW, 1 cycle/row vs fp32's 4); E and the AV matmul run in bf16.
Measured: absmax error vs fp32 reference 6.5e-3 (scale ~1), HW exec time
~244 us/core (PE issue-limited end to end, ~79% of 78.6 TF/s peak).

Pipelining: (1) phases B1/B2 run as two kt-half passes (PSUM partials
drained to SBUF, second pass adds in place) so the v-projection's first
half overlaps the xt4-7/wv DMA stream and the PE never stalls on input
DMA after its first tile; (2) phase C interleaves block n's scores+exp
(per t-tile) into block n-1's first AV accumulation loop so the PE array
never sees a low-density stretch (keeps the HAM clock gate at K=8/8);
block 0's scores ride inside the v-projection tail.
"""

import numpy as np

import concourse.bacc as bacc
import concourse.mybir as mybir
import concourse.tile as tile
from concourse.bass_utils import run_bass_kernel_spmd

B, S, IN, QD, VD = 8, 2048, 1024, 128, 1024
N_CORES = 8
P = 128
KT = IN // P          # 8 contraction tiles for projections
TT = S // P           # 16 t-tiles
S_BLK = 512           # s-block width for scores/E^T staging
N_BLK = S // S_BLK    # 8 blocks
SS = S_BLK // P       # 2 s-subtiles per block

F32 = mybir.dt.float32
F32R = mybir.dt.float32r
BF16 = mybir.dt.bfloat16

_CACHE: dict = {}


def _build():
    if "nc" in _CACHE:
        return _CACHE["nc"]

    nc = bacc.Bacc("TRN2", target_bir_lowering=False, debug=False,
                   num_devices=N_CORES)

    xT_d = nc.dram_tensor("xT", [IN, S], F32, kind="ExternalInput").ap()
    wq_d = nc.dram_tensor("wq", [P, KT, QD], F32, kind="ExternalInput").ap()
    wk_d = nc.dram_tensor("wk", [P, KT, QD], F32, kind="ExternalInput").ap()
    wv_d = nc.dram_tensor("wv", [P, KT, VD], F32, kind="ExternalInput").ap()
    bq_d = nc.dram_tensor("bq", [QD], F32, kind="ExternalInput").ap()
    bk_d = nc.dram_tensor("bk", [QD], F32, kind="ExternalInput").ap()
    bv_d = nc.dram_tensor("bv", [VD], F32, kind="ExternalInput").ap()
    out_d = nc.dram_tensor("out", [S, VD], F32, kind="ExternalOutput").ap()

    with tile.TileContext(nc) as tc:
        with (
            tc.tile_pool(name="consts", bufs=1) as consts,
            tc.tile_pool(name="xt", bufs=KT) as p_xt,
            tc.tile_pool(name="wv", bufs=KT) as p_wv,
            tc.tile_pool(name="qk", bufs=1) as p_qk,
            tc.tile_pool(name="v", bufs=TT) as p_v,
            tc.tile_pool(name="et", bufs=2 * TT) as p_et,
            tc.tile_pool(name="o", bufs=2) as p_o,
            tc.tile_pool(name="recip", bufs=4) as p_recip,
            tc.tile_pool(name="ps", bufs=8, space="PSUM") as ps,
        ):
            # ---- constant / weight loads ----
            wq_sb = consts.tile([P, KT, QD], F32R, tag="wq")
            wk_sb = consts.tile([P, KT, QD], F32R, tag="wk")
            nc.sync.dma_start(out=wq_sb[:], in_=wq_d.bitcast(F32R))
            nc.sync.dma_start(out=wk_sb[:], in_=wk_d.bitcast(F32R))

            def wq_at(kt):
                return wq_sb[:, kt, :]

            def wk_at(kt):
                return wk_sb[:, kt, :]

            ones_sb = consts.tile([P, 1], BF16, tag="ones")
            nc.vector.memset(ones_sb[:], 1.0)

            xt_sb = []
            bq_sb = consts.tile([P, 1], F32, tag="bq")
            bk_sb = consts.tile([P, 1], F32, tag="bk")
            bv_row = consts.tile([1, VD], F32, tag="bv_row")
            bv_sb = consts.tile([P, VD], F32, tag="bv")
            wv_sb = [None] * KT

            def load_wv(kt):
                t_ = p_wv.tile([P, VD], F32R, tag="wv", name=f"wvt{kt}")
                nc.sync.dma_start(out=t_[:], in_=wv_d[:, kt, :].bitcast(F32R))
                wv_sb[kt] = t_

            for kt in range(KT):
                t_ = p_xt.tile([P, S], F32R, tag="xt", name=f"xt{kt}")
                nc.sync.dma_start(out=t_[:],
                                  in_=xT_d[kt * P:(kt + 1) * P, :].bitcast(F32R))
                xt_sb.append(t_)
                if kt == KT // 2 - 1:
                    # wv0-3 between xt3 and xt4: needed by the first
                    # v-projection half-pass
                    for wkt in range(KT // 2):
                        load_wv(wkt)
                if kt == 0:
                    # small loads tucked behind xt0 so they don't delay it
                    # but still land long before their first use
                    nc.sync.dma_start(out=bq_sb[:],
                                      in_=bq_d.rearrange("(p o) -> p o", o=1))
                    nc.sync.dma_start(out=bk_sb[:],
                                      in_=bk_d.rearrange("(p o) -> p o", o=1))
                    nc.sync.dma_start(out=bv_row[:],
                                      in_=bv_d.rearrange("(o v) -> o v", o=1))
                    # broadcast bv across partitions via a K=1 outer product
                    # (ones[1,P] x bv[1,VD]) - no HBM bandwidth stolen from
                    # the xt/wv input stream
                    ones_row = consts.tile([1, P], BF16, tag="ones_row")
                    nc.vector.memset(ones_row[:], 1.0)
                    bv_row_bf = consts.tile([1, VD], BF16, tag="bv_row_bf")
                    nc.vector.tensor_copy(bv_row_bf[:], bv_row[:])
                    for c in range(VD // 512):
                        bv_ps = ps.tile([P, 512], F32, tag="ps",
                                        name=f"bvps{c}")
                        nc.tensor.matmul(bv_ps[:], ones_row[:],
                                         bv_row_bf[:, c * 512:(c + 1) * 512],
                                         start=True, stop=True)
                        nc.vector.tensor_copy(bv_sb[:, c * 512:(c + 1) * 512],
                                              bv_ps[:])

            # ---- phases B1/B2 as two kt-half passes ----
            # Projections accumulate kt 0-3 into PSUM, drain partials to
            # SBUF, then a second pass adds kt 4-7 in place. Freeing all 8
            # PSUM banks between passes lets the v-projection's first half
            # (which only needs xt0-3 + wv0-3) run while xt4-7 / wv4-7 are
            # still streaming in, so the PE never waits on the input DMA
            # after its first tile.
            qT_sb = p_qk.tile([P, S], F32R, tag="qT")
            kT_sb = p_qk.tile([P, S], F32R, tag="kT")
            NSC = S // 512  # 4
            NVC = VD // 512
            KH = KT // 2

            def proj_pass(half):
                k0 = half * KH
                q_ps = [ps.tile([P, 512], F32, tag="ps",
                                name=f"qps{half}_{i}") for i in range(NSC)]
                k_ps = [ps.tile([P, 512], F32, tag="ps",
                                name=f"kps{half}_{i}") for i in range(NSC)]
                for kt in range(k0, k0 + KH):
                    # all q chunks then all k chunks: one weight load per
                    # group instead of one per matmul
                    for sc in range(NSC):
                        nc.tensor.matmul(q_ps[sc][:], wq_at(kt),
                                         xt_sb[kt][:, sc * 512:(sc + 1) * 512],
                                         start=(kt == k0),
                                         stop=(kt == k0 + KH - 1))
                    for sc in range(NSC):
                        nc.tensor.matmul(k_ps[sc][:], wk_at(kt),
                                         xt_sb[kt][:, sc * 512:(sc + 1) * 512],
                                         start=(kt == k0),
                                         stop=(kt == k0 + KH - 1))
                for sc in range(NSC):
                    sl = slice(sc * 512, (sc + 1) * 512)
                    if half == 0:
                        nc.vector.tensor_scalar_add(qT_sb[:, sl], q_ps[sc][:],
                                                    bq_sb[:])
                        nc.vector.tensor_scalar_add(kT_sb[:, sl], k_ps[sc][:],
                                                    bk_sb[:])
                    else:
                        nc.vector.tensor_add(qT_sb[:, sl], q_ps[sc][:],
                                             qT_sb[:, sl])
                        nc.vector.tensor_add(kT_sb[:, sl], k_ps[sc][:],
                                             kT_sb[:, sl])

            proj_pass(0)

            # ---- phase C helper (defined early: scores for block 0 are
            # interleaved into phase B2's tail) ----
            def emit_scores_t(sb, t):
                s0 = sb * S_BLK
                st_ps = ps.tile([P, S_BLK], F32, tag="ps", name=f"stps{sb}_{t}")
                nc.tensor.matmul(st_ps[:],
                                 kT_sb[:, t * P:(t + 1) * P],
                                 qT_sb[:, s0:s0 + S_BLK],
                                 start=True, stop=True)
                et = p_et.tile([P, S_BLK], BF16, tag="et", name=f"et{sb}_{t}")
                nc.scalar.activation(out=et[:], in_=st_ps[:],
                                     func=mybir.ActivationFunctionType.Exp)
                return et

            # ---- phase B2: v [S, VD] = xT^T Wv + bv, stored bf16 ----
            # Two kt-half passes; pass 0 stores bf16 partials (+bv) in v_sb,
            # pass 1 adds the kt 4-7 contribution in place. Block 0's 16
            # scores/exp tiles ride along in pass 1's last iterations so
            # phase C starts with E^T(0) already staged.
            v_sb = [p_v.tile([P, VD], BF16, tag="v", name=f"v{t}")
                    for t in range(TT)]
            et0 = []

            def v_pass(half, interleave0):
                k0 = half * KH
                for t in range(TT):
                    vt = v_sb[t]
                    if interleave0 and t >= TT - 8:
                        et0.append(emit_scores_t(0, len(et0)))
                    v_ps = [ps.tile([P, 512], F32, tag="ps",
                                    name=f"vps{half}_{t}_{vc}")
                            for vc in range(NVC)]
                    for kt in range(k0, k0 + KH):
                        xl = xt_sb[kt][:, t * P:(t + 1) * P]
                        for vc in range(NVC):
                            nc.tensor.matmul(
                                v_ps[vc][:], xl,
                                wv_sb[kt][:, vc * 512:(vc + 1) * 512],
                                start=(kt == k0), stop=(kt == k0 + KH - 1))
                    if interleave0 and t >= TT - 8:
                        et0.append(emit_scores_t(0, len(et0)))
                    for vc in range(NVC):
                        sl = slice(vc * 512, (vc + 1) * 512)
                        if half == 0:
                            nc.vector.tensor_add(vt[:, sl], v_ps[vc][:],
                                                 bv_sb[:, sl])
                        else:
                            nc.vector.tensor_add(vt[:, sl], v_ps[vc][:],
                                                 vt[:, sl])

            v_pass(0, interleave0=False)
            for kt in range(KT // 2, KT):
                load_wv(kt)
            proj_pass(1)
            v_pass(1, interleave0=True)

            # ---- phase C: software-pipelined over s-blocks ----
            # Block n's scores^T + exp are interleaved (per t) into block
            # n-1's first AV accumulation loop so the PE array never sees a
            # low-density stretch (keeps HAM at K=8/8) and exp latency hides
            # under the AV matmul stream.
            def emit_av_ss(sb, ss, et_tiles, interleave_sb=None,
                           serialize_vc=False):
                # One AV accumulation group (128 output rows x full VD) plus
                # its denominator; optionally interleaves the next block's
                # scores/exp into the t loop.
                o_ps = [ps.tile([P, 512], F32, tag="ps", name=f"ops{sb}_{ss}_{i}")
                        for i in range(VD // 512)]
                d_ps = ps.tile([P, 1], F32, tag="ps", name=f"dps{sb}_{ss}")
                nxt = []
                recip = p_recip.tile([P, 1], F32, tag="recip",
                                     name=f"recip{sb}_{ss}")
                o_sb = p_o.tile([P, VD], F32, tag="o", name=f"osb{sb}_{ss}")
                srow = sb * S_BLK + ss * P

                def drain_vc(vc):
                    nc.scalar.activation(
                        out=o_sb[:, vc * 512:(vc + 1) * 512],
                        in_=o_ps[vc][:],
                        func=mybir.ActivationFunctionType.Tanh,
                        scale=recip[:])
                    nc.sync.dma_start(
                        out=out_d[srow:srow + P, vc * 512:(vc + 1) * 512],
                        in_=o_sb[:, vc * 512:(vc + 1) * 512])

                if not serialize_vc:
                    for t in range(TT):
                        if interleave_sb is not None:
                            nxt.append(emit_scores_t(interleave_sb, t))
                        lhs = et_tiles[t][:, ss * P:(ss + 1) * P]
                        for vc in range(VD // 512):
                            nc.tensor.matmul(o_ps[vc][:], lhs,
                                             v_sb[t][:, vc * 512:(vc + 1) * 512],
                                             start=(t == 0), stop=(t == TT - 1))
                        nc.tensor.matmul(d_ps[:], lhs, ones_sb[:],
                                         start=(t == 0), stop=(t == TT - 1))
                    nc.vector.reciprocal(recip[:], d_ps[:])
                    for vc in range(VD // 512):
                        drain_vc(vc)
                else:
                    # tail variant: finish vc0 (and the denominator) first so
                    # its tanh+DMA overlap vc1's accumulation
                    for t in range(TT):
                        lhs = et_tiles[t][:, ss * P:(ss + 1) * P]
                        nc.tensor.matmul(o_ps[0][:], lhs, v_sb[t][:, 0:512],
                                         start=(t == 0), stop=(t == TT - 1))
                        nc.tensor.matmul(d_ps[:], lhs, ones_sb[:],
                                         start=(t == 0), stop=(t == TT - 1))
                    nc.vector.reciprocal(recip[:], d_ps[:])
                    drain_vc(0)
                    for t in range(TT):
                        lhs = et_tiles[t][:, ss * P:(ss + 1) * P]
                        nc.tensor.matmul(o_ps[1][:], lhs,
                                         v_sb[t][:, 512:1024],
                                         start=(t == 0), stop=(t == TT - 1))
                    drain_vc(1)
                return nxt

            et_cur = et0
            for sb in range(N_BLK):
                nxt_sb = sb + 1 if sb + 1 < N_BLK else None
                et_nxt = emit_av_ss(sb, 0, et_cur, interleave_sb=nxt_sb)
                for ssi in range(1, SS):
                    emit_av_ss(sb, ssi, et_cur,
                               serialize_vc=(sb == N_BLK - 1
                                             and ssi == SS - 1))
                et_cur = et_nxt

    nc.compile()
    _CACHE["nc"] = nc
    return nc


def _prep_inputs(x, Wq, bq, Wk, bk, Wv, bv):
    x = np.asarray(x, np.float32)
    xT = np.ascontiguousarray(x.transpose(0, 2, 1))          # [B, IN, S]
    wq = np.ascontiguousarray(
        np.asarray(Wq, np.float32).reshape(KT, P, QD).transpose(1, 0, 2))
    wk = np.ascontiguousarray(
        np.asarray(Wk, np.float32).reshape(KT, P, QD).transpose(1, 0, 2))
    wv = np.ascontiguousarray(
        np.asarray(Wv, np.float32).reshape(KT, P, VD).transpose(1, 0, 2))
    shared = {
        "wq": wq, "wk": wk, "wv": wv,
        "bq": np.asarray(bq, np.float32),
        "bk": np.asarray(bk, np.float32),
        "bv": np.asarray(bv, np.float32),
    }
    return [dict(shared, xT=xT[c]) for c in range(N_CORES)]


def run(x, Wq, bq, Wk, bk, Wv, bv, trace=False):
    nc = _build()
    in_maps = _prep_inputs(x, Wq, bq, Wk, bk, Wv, bv)
    res = run_bass_kernel_spmd(nc, in_maps, list(range(N_CORES)), trace=trace)
    out = np.stack([res.results[c]["out"] for c in range(N_CORES)])
    return out.astype(np.float32), res


def kernel(x, Wq, bq, Wk, bk, Wv, bv):
    out, _ = run(x, Wq, bq, Wk, bk, Wv, bv, trace=False)
    return out
